# revision 31
# baseline (speedup 1.0000x reference)
"""Trainium2 Bass kernel for DimPositionalEmbedding (odometer positional embedding).

Computes, per batch sequence (8 sequences x 4096 tokens, one NeuronCore each):
  - a mixed-radix "odometer" counter scan over tokens (EOS token 1 freezes,
    token 10 bumps dim1, token 11 bumps dim2, others bump dim0; radices
    [514, 256, 128] with carry propagation),
  - a 3-table embedding gather + sum: emb0[c0] + emb1[c1] + emb2[c2].

The sequential scan is reformulated as prefix sums / prefix maxes (validated
bitwise against the reference):
  d      = prefix_or(tok == 1)              (done)
  m      = 1 - d                            (alive)
  ev1    = (tok == 10) & m ; ev2 = (tok == 11) & m
  i0     = m - ev1 - ev2                    (ordinary & alive)
  cum0'  = 2 + prefix_sum(i0)
  A'     = prefix_max(cum0' * (ev1|ev2))
  c0     = (cum0' - A') mod 514
  carry0 = (c0 == 0) * i0
  cumI1  = prefix_sum(ev1 + carry0)
  B      = prefix_max(cumI1 * ev2)
  c1     = (cumI1 - B) mod 256
  carry1 = (c1 == 0) * (ev1 + carry0)
  c2     = prefix_sum(ev2 + carry1) mod 128
  emap_j = d ? MAXL_j - 1 : c_j
Prefix ops run on the Vector engine as two-level scans: tensor_tensor_scan
along the free dim of a [32, 128] layout (partition p owns tokens
[128p, 128p+128)), then a 32-wide cross-partition stitch via 32x32 vector
transposes.

Gathers run as indirect (SWDGE) DMAs from HBM with CCE fp32 accumulate, so
the three table lookups and the two adds all happen inside the DMA datapath.
"""

import sys

for _p in ("/opt/trn_rl_repo", "/opt/pypackages"):
    if _p not in sys.path:
        sys.path.append(_p)

import numpy as np
from concourse import bass, bacc, mybir
from concourse.bass_utils import run_bass_kernel_spmd
from concourse.tile import TileContext
from concourse.masks import make_identity

F32 = mybir.dt.float32
I32 = mybir.dt.int32
ALU = mybir.AluOpType

S = 4096          # sequence length
D = 1024          # embedding dim
NP = 32           # scan partitions (chunks)
NF = 128          # tokens per chunk (scan free dim)
NCORES = 8
V0, V1, V2 = 514, 256, 128   # radices / table sizes


def _scan_phase(nc, sp, pp, tokI, tagp):
    """Emit the odometer scan; returns (idxT list of [128,32] i32 emap
    transposes, (c0, c1, c2) scan tiles)."""
    _uid = [0]

    def tg(t):
        _uid[0] += 1
        return f"{tagp}{t}{_uid[0]}"

    Z = sp.tile([NP, NF], F32, tag=tg("Z"))
    nc.vector.memset(Z[:], 0.0)

    def series(X, op, init, tag):
        W = sp.tile([NP, NF], F32, tag=tg(tag))
        nc.vector.tensor_tensor_scan(
            out=W[:], data0=X[:], data1=Z[:], initial=0.0, op0=op, op1=ALU.add
        )
        T = sp.tile([NP, NP], F32, tag=tg(tag))
        nc.vector.memset(T[:], 0.0)
        nc.vector.tensor_copy(out=T[:, 0:1], in_=W[:, NF - 1 : NF])
        TT = sp.tile([NP, NP], F32, tag=tg(tag))
        nc.vector.transpose(out=TT[:], in_=T[:])
        SS = sp.tile([NP, NP], F32, tag=tg(tag))
        nc.vector.memset(SS[:], 0.0)
        if init != 0.0:
            nc.vector.memset(SS[0:1, 0:1], init)
        nc.vector.tensor_tensor_scan(
            out=SS[0:1, 1:NP], data0=TT[0:1, 0 : NP - 1],
            data1=Z[0:1, 0 : NP - 1], initial=float(init), op0=op, op1=ALU.add,
        )
        SB = sp.tile([NP, NP], F32, tag=tg(tag))
        nc.vector.transpose(out=SB[:], in_=SS[:])
        P = sp.tile([NP, NF], F32, tag=tg(tag))
        nc.vector.tensor_scalar(
            out=P[:], in0=W[:], scalar1=SB[:, 0:1], scalar2=None, op0=op
        )
        return P

    def mod_levels(X, levels, tag):
        cur = X
        for K in levels:
            ge = sp.tile([NP, NF], F32, tag=tg(tag))
            nc.vector.tensor_scalar(
                out=ge[:], in0=cur[:], scalar1=float(K), scalar2=None,
                op0=ALU.is_ge,
            )
            nxt = sp.tile([NP, NF], F32, tag=tg(tag))
            nc.vector.scalar_tensor_tensor(
                out=nxt[:], in0=ge[:], scalar=float(-K), in1=cur[:],
                op0=ALU.mult, op1=ALU.add,
            )
            cur = nxt
        return cur

    def ts(in0, s1, op0, s2=None, op1=None, tag="t"):
        o = sp.tile([NP, NF], F32, tag=tg(tag))
        nc.vector.tensor_scalar(
            out=o[:], in0=in0[:], scalar1=s1, scalar2=s2, op0=op0,
            **({"op1": op1} if op1 is not None else {}),
        )
        return o

    def tt(in0, in1, op, tag="t"):
        o = sp.tile([NP, NF], F32, tag=tg(tag))
        nc.vector.tensor_tensor(out=o[:], in0=in0[:], in1=in1[:], op=op)
        return o

    def stt(in0, s, in1, op0, op1, tag="t"):
        o = sp.tile([NP, NF], F32, tag=tg(tag))
        nc.vector.scalar_tensor_tensor(
            out=o[:], in0=in0[:], scalar=float(s), in1=in1[:], op0=op0, op1=op1,
        )
        return o

    e = ts(tokI, 1, ALU.is_equal)
    k1r = ts(tokI, 10, ALU.is_equal)
    k2r = ts(tokI, 11, ALU.is_equal)
    cntE = series(e, ALU.add, 0.0, "dE")
    d = ts(cntE, 1.0, ALU.is_ge)
    fe = stt(cntE, 1.0, e, ALU.is_equal, ALU.mult)  # first-EOS indicator
    m = ts(d, -1.0, ALU.mult, 1.0, ALU.add)
    ev1 = tt(k1r, m, ALU.mult)
    ev2 = tt(k2r, m, ALU.mult)
    evA = tt(ev1, ev2, ALU.add)
    i0 = tt(m, evA, ALU.subtract)
    cum0p = series(i0, ALU.add, 2.0, "c0s")
    Amask = tt(cum0p, evA, ALU.mult)
    Ap = series(Amask, ALU.max, 0.0, "As")
    x0 = tt(cum0p, Ap, ALU.subtract)
    c0 = mod_levels(x0, [2056, 1028, 514], "m0")
    carry0 = stt(c0, 0.0, i0, ALU.is_equal, ALU.mult)
    inc1 = tt(ev1, carry0, ALU.add)
    cumI1 = series(inc1, ALU.add, 0.0, "c1s")
    Bmask = tt(cumI1, ev2, ALU.mult)
    B = series(Bmask, ALU.max, 0.0, "Bs")
    x1 = tt(cumI1, B, ALU.subtract)
    c1 = mod_levels(x1, [2048, 1024, 512, 256], "m1")
    carry1 = stt(c1, 0.0, inc1, ALU.is_equal, ALU.mult)
    inc2 = tt(ev2, carry1, ALU.add)
    cumI2 = series(inc2, ALU.add, 0.0, "c2s")
    c2 = mod_levels(cumI2, [2048, 1024, 512, 256, 128], "m2")

    # (c1, c2)-pair segment ids: a new segment starts at every event that can
    # change the pair (ev1/ev2/carry0 are mutually exclusive) or at first EOS
    chg = tt(evA, carry0, ALU.add)
    chg = tt(chg, fe, ALU.add)
    jseg = series(chg, ALU.add, 0.0, "js")

    # emap_j = c_j * m + (MAXL_j - 1) * d, transposed to [128, 32] int32
    ident = sp.tile([NP, NP], F32, tag=tg("id"))
    make_identity(nc, ident[:])
    idxT = []
    idxTf = []
    for j, (c, mx) in enumerate(((c0, V0), (c1, V1), (c2, V2))):
        em = tt(c, m, ALU.mult)
        em = stt(d, float(mx - 1), em, ALU.mult, ALU.add)
        pt = pp.tile([NF, NP], F32, space="PSUM", tag="pt_shared")
        nc.tensor.transpose(out=pt[:], in_=em[:], identity=ident[:])
        ix = sp.tile([NF, NP], I32, tag=tg("ix"))
        nc.vector.tensor_copy(out=ix[:], in_=pt[:])
        idxT.append(ix)
        ixf = sp.tile([NF, NP], F32, tag=tg("ixf"))
        nc.vector.tensor_copy(out=ixf[:], in_=pt[:])
        idxTf.append(ixf)
    # transposed segment ids and segment-start mask (fp32)
    jT = sp.tile([NF, NP], F32, tag=tg("jT"))
    chgT = sp.tile([NF, NP], F32, tag=tg("chgT"))
    for src, dst in ((jseg, jT), (chg, chgT)):
        pt = pp.tile([NF, NP], F32, space="PSUM", tag="pt_shared")
        nc.tensor.transpose(out=pt[:], in_=src[:], identity=ident[:])
        nc.vector.tensor_copy(out=dst[:], in_=pt[:])
    return idxT, idxTf, jT, chgT, (c0, c1, c2)


def _build_body(nc, sp, op_, pp, ids, emb0, emb1, emb2, pos, fc, rep,
                do_scan=True, gathers=(0, 1, 2), store=True, gather_cols=1,
                pe_pairs=False):
    tagp = ""  # shared tags across reps -> reps reuse the same SBUF slots
    tokI = sp.tile([NP, NF], I32, tag=f"{tagp}tokI")
    nc.sync.dma_start(out=tokI[:], in_=ids.ap()[:])

    if do_scan:
        idxT, idxTf, jT, chgT, (c0, c1, c2) = _scan_phase(nc, sp, pp, tokI,
                                                          tagp)
        # final counters: c0/c1/c2 at token 4095 = [31, 127]
        # (DMA has no partition-start limits; cast to int32 on host)
        for j, c in enumerate((c0, c1, c2)):
            nc.sync.dma_start(
                out=fc.ap()[0:1, j : j + 1], in_=c[31:32, NF - 1 : NF]
            )
        if not gathers and not store and not pe_pairs:
            return
    else:
        # ablation: fake indices derived trivially from tokens
        ix = sp.tile([NF, NP], I32, tag="fakeix")
        nc.gpsimd.iota(ix[:], pattern=[[0, NP]], base=0, channel_multiplier=1)
        ixf = sp.tile([NF, NP], F32, tag="fakeixf")
        nc.vector.tensor_copy(out=ixf[:], in_=ix[:])
        idxT = [ix, ix, ix]
        idxTf = [ixf, ixf, ixf]
        jT = chgT = ixf

    if pe_pairs:
        identF = sp.tile([NF, NF], F32, tag="identF")
        make_identity(nc, identF[:])
        iotaI = sp.tile([NF, 1], I32, tag="iotaI")
        nc.gpsimd.iota(iotaI[:], pattern=[[0, 1]], base=0, channel_multiplier=1)
        iotaF = sp.tile([NF, 1], F32, tag="iotaF")
        nc.vector.tensor_copy(out=iotaF[:], in_=iotaI[:])
        iotaRowI = sp.tile([NF, NF], I32, tag="iotaRowI")
        nc.gpsimd.iota(iotaRowI[:], pattern=[[1, NF]], base=0,
                       channel_multiplier=0)
        iotaRow = sp.tile([NF, NF], F32, tag="iotaRow")
        nc.vector.tensor_copy(out=iotaRow[:], in_=iotaRowI[:])

        # scatter (c1,c2) of each segment's first token into pc[seg] via 32
        # accumulating one-hot matmuls (exact in fp32). The two accumulation
        # groups sit in different PSUM banks (cols 0 and 512); the tile
        # shares the scan-transpose slot (sequential lifetimes).
        pc = pp.tile([NF, D], F32, space="PSUM", tag="pt_shared")
        for b in range(NP):
            ohs = sp.tile([NF, NF], F32, tag=f"ohs{b % 2}")
            nc.vector.scalar_tensor_tensor(
                out=ohs[:], in0=iotaRow[:], scalar=jT[:, b : b + 1],
                in1=chgT[:, b : b + 1].to_broadcast([NF, NF]),
                op0=ALU.is_equal, op1=ALU.mult,
            )
            nc.tensor.matmul(pc[:, 0:1], lhsT=ohs[:],
                             rhs=idxTf[1][:, b : b + 1],
                             start=(b == 0), stop=(b == NP - 1))
            nc.tensor.matmul(pc[:, 512:513], lhsT=ohs[:],
                             rhs=idxTf[2][:, b : b + 1],
                             start=(b == 0), stop=(b == NP - 1))
        pcI = sp.tile([NF, 2], I32, tag="pcI")
        nc.vector.tensor_copy(out=pcI[:, 0:1], in_=pc[:, 0:1])
        nc.vector.tensor_copy(out=pcI[:, 1:2], in_=pc[:, 512:513])
        # pair table: pairsum[j, :] = emb1[pc1_j] + emb2[pc2_j]
        pairsum = sp.tile([NF, D], F32, tag="pairsum")
        nc.gpsimd.indirect_dma_start(
            out=pairsum[:], out_offset=None, in_=emb1.ap()[:],
            in_offset=bass.IndirectOffsetOnAxis(ap=pcI[:, 0:1], axis=0),
        )
        nc.gpsimd.indirect_dma_start(
            out=pairsum[:], out_offset=None, in_=emb2.ap()[:],
            in_offset=bass.IndirectOffsetOnAxis(ap=pcI[:, 1:2], axis=0),
            compute_op=ALU.add,
        )

    tabs = (emb0, emb1, emb2)
    k = gather_cols
    for b in range(0, NP, k):
        t = op_.tile([NF, k * D], F32, tag="out")
        first = True
        for j in gathers:
            nc.gpsimd.indirect_dma_start(
                out=t[:], out_offset=None, in_=tabs[j].ap()[:],
                in_offset=bass.IndirectOffsetOnAxis(
                    ap=idxT[j][:, b : b + k], axis=0),
                compute_op=ALU.bypass if first else ALU.add,
            )
            first = False
        if first and not pe_pairs:
            nc.vector.memset(t[:], 0.0)

        if pe_pairs:
            # broadcast segment ids across partitions via PE transpose, build
            # the segment one-hot lhsT on DVE, then po = pairsum[jseg_tok]
            ctj = _build_body._ct_pool.tile([NF, NF], F32, space="PSUM",
                                            tag="ctj")
            nc.tensor.transpose(
                out=ctj[:], in_=jT[:, b : b + 1].to_broadcast([NF, NF]),
                identity=identF[:],
            )
            ohj = sp.tile([NF, NF], F32, tag=f"ohj{b % 2}")
            nc.vector.tensor_tensor(
                out=ohj[:], in0=ctj[:],
                in1=iotaF[:].to_broadcast([NF, NF]), op=ALU.is_equal,
            )
            po = _build_body._po_pool.tile([NF, D], F32, space="PSUM", tag="po")
            for h in range(0, D, 512):
                sl = slice(h, h + 512)
                nc.tensor.matmul(po[:, sl], lhsT=ohj[:], rhs=pairsum[:, sl],
                                 start=True, stop=True)
            if gathers:
                nc.vector.tensor_tensor(
                    out=t[:], in0=t[:], in1=po[:], op=ALU.add
                )
            else:
                nc.vector.tensor_copy(out=t[:], in_=po[:])

        if store:
            if k == 1:
                nc.sync.dma_start(
                    out=pos.ap()[b * NF : (b + 1) * NF, :], in_=t[:]
                )
            else:
                dst = pos.ap()[b * NF : (b + k) * NF, :].rearrange(
                    "(k p) d -> p k d", p=NF
                )
                nc.sync.dma_start(
                    out=dst, in_=t[:].rearrange("p (k d) -> p k d", k=k)
                )


def build(out_bufs=6, reps=1, **variant):
    nc = bacc.Bacc("TRN2", target_bir_lowering=False, debug=False,
                   num_devices=NCORES)
    ids = nc.dram_tensor("ids", [NP, NF], I32, kind="ExternalInput")
    emb0 = nc.dram_tensor("emb0", [V0, D], F32, kind="ExternalInput")
    emb1 = nc.dram_tensor("emb1", [V1, D], F32, kind="ExternalInput")
    emb2 = nc.dram_tensor("emb2", [V2, D], F32, kind="ExternalInput")
    pos = nc.dram_tensor("pos", [S, D], F32, kind="ExternalOutput")
    fc = nc.dram_tensor("fc", [1, 3], F32, kind="ExternalOutput")

    with TileContext(nc) as tc:
        with (
            tc.tile_pool(name="scan", bufs=1) as sp,
            tc.tile_pool(name="outp", bufs=out_bufs) as op_,
            tc.tile_pool(name="psum", bufs=1, space="PSUM") as pp,
            tc.tile_pool(name="psum_po", bufs=2, space="PSUM") as pp2,
            tc.tile_pool(name="psum_ct", bufs=2, space="PSUM") as pp3,
        ):
            _build_body._po_pool = pp2
            _build_body._ct_pool = pp3
            for rep in range(reps):
                _build_body(nc, sp, op_, pp, ids, emb0, emb1, emb2, pos, fc,
                            rep, **variant)

    nc.compile()
    return nc


_NC_CACHE = {}

BEST_VARIANT = dict(gathers=(0,), pe_pairs=True)
FALLBACK_VARIANT = dict()  # all-DMA gather path, no segment cap


def _get_nc(fallback=False):
    key = "fb" if fallback else "best"
    if key not in _NC_CACHE:
        _NC_CACHE[key] = build(**(FALLBACK_VARIANT if fallback
                                  else BEST_VARIANT))
    return _NC_CACHE[key]


def _max_segments(ids):
    """Host replica of the segment count: the pe_pairs path supports at most
    128 (c1,c2)-segments per sequence."""
    mx = 0
    for tok in ids:
        e = (tok == 1)
        dn = np.maximum.accumulate(e)
        m = ~dn
        k1 = (tok == 10) & m
        k2 = (tok == 11) & m
        i0 = (~(tok == 10)) & (~(tok == 11)) & m
        cum0p = 2 + np.cumsum(i0)
        evA = k1 | k2
        Ap = np.maximum.accumulate(cum0p * evA)
        c0 = (cum0p - Ap) % 514
        carry0 = (c0 == 0) & i0
        nseg = int(k1.sum() + k2.sum() + carry0.sum() + e.any())
        mx = max(mx, nseg)
    return mx


def kernel(input_ids, emb0, emb1, emb2, _nc=None):
    ids = np.ascontiguousarray(np.asarray(input_ids).astype(np.int32))
    emb0 = np.ascontiguousarray(np.asarray(emb0, dtype=np.float32))
    emb1 = np.ascontiguousarray(np.asarray(emb1, dtype=np.float32))
    emb2 = np.ascontiguousarray(np.asarray(emb2, dtype=np.float32))
    assert ids.shape == (NCORES, S)

    if _nc is not None:
        nc = _nc
    else:
        nc = _get_nc(fallback=_max_segments(ids) > NF - 1)
    in_maps = [
        {
            "ids": ids[i].reshape(NP, NF),
            "emb0": emb0,
            "emb1": emb1,
            "emb2": emb2,
        }
        for i in range(NCORES)
    ]
    res = run_bass_kernel_spmd(nc, in_maps, core_ids=list(range(NCORES)))
    pos = np.stack([res.results[i]["pos"] for i in range(NCORES)])
    fc = np.stack([res.results[i]["fc"].reshape(3) for i in range(NCORES)])
    fc = np.rint(fc).astype(np.int32)
    return pos, fc


# revision 47
# speedup vs baseline: 24.3336x; 24.3336x over previous
"""Trainium2 Bass kernel for DimPositionalEmbedding (odometer positional embedding).

Computes, per batch sequence (8 sequences x 4096 tokens, one NeuronCore each):
  - a mixed-radix "odometer" counter scan over tokens (EOS token 1 freezes,
    token 10 bumps dim1, token 11 bumps dim2, others bump dim0; radices
    [514, 256, 128] with carry propagation),
  - a 3-table embedding gather + sum: emb0[c0] + emb1[c1] + emb2[c2].

The sequential scan is reformulated as prefix sums / prefix maxes (validated
bitwise against the reference):
  d      = prefix_or(tok == 1)              (done)
  m      = 1 - d                            (alive)
  ev1    = (tok == 10) & m ; ev2 = (tok == 11) & m
  i0     = m - ev1 - ev2                    (ordinary & alive)
  cum0'  = 2 + prefix_sum(i0)
  A'     = prefix_max(cum0' * (ev1|ev2))
  c0     = (cum0' - A') mod 514
  carry0 = (c0 == 0) * i0
  cumI1  = prefix_sum(ev1 + carry0)
  B      = prefix_max(cumI1 * ev2)
  c1     = (cumI1 - B) mod 256
  carry1 = (c1 == 0) * (ev1 + carry0)
  c2     = prefix_sum(ev2 + carry1) mod 128
  emap_j = d ? MAXL_j - 1 : c_j
Prefix ops run on the Vector engine as two-level scans: tensor_tensor_scan
along the free dim of a [32, 128] layout (partition p owns tokens
[128p, 128p+128)), then a 32-wide cross-partition stitch via 32x32 vector
transposes.

Gathers run as indirect (SWDGE) DMAs from HBM with CCE fp32 accumulate, so
the three table lookups and the two adds all happen inside the DMA datapath.
"""

import sys

for _p in ("/opt/trn_rl_repo", "/opt/pypackages"):
    if _p not in sys.path:
        sys.path.append(_p)

import numpy as np
from concourse import bass, bacc, mybir
from concourse.bass_utils import run_bass_kernel_spmd
from concourse.tile import TileContext
from concourse.masks import make_identity

F32 = mybir.dt.float32
I32 = mybir.dt.int32
ALU = mybir.AluOpType

S = 4096          # sequence length
D = 1024          # embedding dim
NP = 32           # scan partitions (chunks)
NF = 128          # tokens per chunk (scan free dim)
NCORES = 8
V0, V1, V2 = 514, 256, 128   # radices / table sizes


def _mod_chain(base, xmax):
    """Minimal conditional-subtract level list reducing x<=xmax to [0, base)."""
    hi = base
    while hi * 2 <= xmax:
        hi *= 2
    levels = []
    while hi >= base:
        levels.append(hi)
        hi //= 2
    return levels


def _scan_phase(nc, sp, pp, tokI, tagp, x1max=4096, x2max=4096,
                pairs=False):
    """Emit the odometer scan; returns (idxT list of [128,32] i32 emap
    transposes, (c0, c1, c2) scan tiles)."""
    _uid = [0]

    def tg(t):
        _uid[0] += 1
        return f"{tagp}{t}{_uid[0]}"

    Z = sp.tile([NP, NF], F32, tag=tg("Z"))
    nc.vector.memset(Z[:], 0.0)

    def series(X, op, init, tag):
        W = sp.tile([NP, NF], F32, tag=tg(tag))
        nc.vector.tensor_tensor_scan(
            out=W[:], data0=X[:], data1=Z[:], initial=0.0, op0=op, op1=ALU.add
        )
        T = sp.tile([NP, NP], F32, tag=tg(tag))
        nc.vector.memset(T[:], 0.0)
        nc.vector.tensor_copy(out=T[:, 0:1], in_=W[:, NF - 1 : NF])
        TT = sp.tile([NP, NP], F32, tag=tg(tag))
        nc.vector.transpose(out=TT[:], in_=T[:])
        SS = sp.tile([NP, NP], F32, tag=tg(tag))
        nc.vector.memset(SS[:], 0.0)
        if init != 0.0:
            nc.vector.memset(SS[0:1, 0:1], init)
        nc.vector.tensor_tensor_scan(
            out=SS[0:1, 1:NP], data0=TT[0:1, 0 : NP - 1],
            data1=Z[0:1, 0 : NP - 1], initial=float(init), op0=op, op1=ALU.add,
        )
        SB = sp.tile([NP, NP], F32, tag=tg(tag))
        nc.vector.transpose(out=SB[:], in_=SS[:])
        P = sp.tile([NP, NF], F32, tag=tg(tag))
        nc.vector.tensor_scalar(
            out=P[:], in0=W[:], scalar1=SB[:, 0:1], scalar2=None, op0=op
        )
        return P

    def mod_levels(X, levels, tag):
        cur = X
        for K in levels:
            ge = sp.tile([NP, NF], F32, tag=tg(tag))
            nc.vector.tensor_scalar(
                out=ge[:], in0=cur[:], scalar1=float(K), scalar2=None,
                op0=ALU.is_ge,
            )
            nxt = sp.tile([NP, NF], F32, tag=tg(tag))
            nc.vector.scalar_tensor_tensor(
                out=nxt[:], in0=ge[:], scalar=float(-K), in1=cur[:],
                op0=ALU.mult, op1=ALU.add,
            )
            cur = nxt
        return cur

    def ts(in0, s1, op0, s2=None, op1=None, tag="t"):
        o = sp.tile([NP, NF], F32, tag=tg(tag))
        nc.vector.tensor_scalar(
            out=o[:], in0=in0[:], scalar1=s1, scalar2=s2, op0=op0,
            **({"op1": op1} if op1 is not None else {}),
        )
        return o

    def tt(in0, in1, op, tag="t"):
        o = sp.tile([NP, NF], F32, tag=tg(tag))
        nc.vector.tensor_tensor(out=o[:], in0=in0[:], in1=in1[:], op=op)
        return o

    def stt(in0, s, in1, op0, op1, tag="t"):
        o = sp.tile([NP, NF], F32, tag=tg(tag))
        nc.vector.scalar_tensor_tensor(
            out=o[:], in0=in0[:], scalar=float(s), in1=in1[:], op0=op0, op1=op1,
        )
        return o

    e = ts(tokI, 1, ALU.is_equal)
    k1r = ts(tokI, 10, ALU.is_equal)
    k2r = ts(tokI, 11, ALU.is_equal)
    cntE = series(e, ALU.add, 0.0, "dE")
    d = ts(cntE, 1.0, ALU.is_ge)
    fe = stt(cntE, 1.0, e, ALU.is_equal, ALU.mult)  # first-EOS indicator
    m = ts(d, -1.0, ALU.mult, 1.0, ALU.add)
    ev1 = tt(k1r, m, ALU.mult)
    ev2 = tt(k2r, m, ALU.mult)
    evA = tt(ev1, ev2, ALU.add)
    i0 = tt(m, evA, ALU.subtract)
    cum0p = series(i0, ALU.add, 2.0, "c0s")
    Amask = tt(cum0p, evA, ALU.mult)
    Ap = series(Amask, ALU.max, 0.0, "As")
    x0 = tt(cum0p, Ap, ALU.subtract)
    c0 = mod_levels(x0, [2056, 1028, 514], "m0")
    carry0 = stt(c0, 0.0, i0, ALU.is_equal, ALU.mult)
    inc1 = tt(ev1, carry0, ALU.add)
    cumI1 = series(inc1, ALU.add, 0.0, "c1s")
    Bmask = tt(cumI1, ev2, ALU.mult)
    B = series(Bmask, ALU.max, 0.0, "Bs")
    x1 = tt(cumI1, B, ALU.subtract)
    c1 = mod_levels(x1, _mod_chain(256, x1max), "m1")
    carry1 = stt(c1, 0.0, inc1, ALU.is_equal, ALU.mult)
    inc2 = tt(ev2, carry1, ALU.add)
    cumI2 = series(inc2, ALU.add, 0.0, "c2s")
    c2 = mod_levels(cumI2, _mod_chain(128, x2max), "m2")

    # (c1, c2)-pair segment ids: a new segment starts at every event that can
    # change the pair (ev1/ev2/carry0 are mutually exclusive) or at first EOS
    chg = tt(evA, carry0, ALU.add)
    chg = tt(chg, fe, ALU.add)
    jseg = series(chg, ALU.add, 0.0, "js")

    # emap_j = c_j * m + (MAXL_j - 1) * d, transposed to [128, 32] int32
    ident = sp.tile([NP, NP], F32, tag=tg("id"))
    make_identity(nc, ident[:])
    idxT = []
    idxTf = []
    for j, (c, mx) in enumerate(((c0, V0), (c1, V1), (c2, V2))):
        em = tt(c, m, ALU.mult)
        em = stt(d, float(mx - 1), em, ALU.mult, ALU.add)
        pt = pp.tile([NF, NP], F32, space="PSUM", tag="pt_shared")
        nc.tensor.transpose(out=pt[:], in_=em[:], identity=ident[:])
        if j == 0 or not pairs:
            ix = sp.tile([NF, NP], I32, tag=tg("ix"))
            nc.vector.tensor_copy(out=ix[:], in_=pt[:])
        else:
            ix = None
        idxT.append(ix)
        if j > 0 and pairs:
            ixf = sp.tile([NF, NP], F32, tag=tg("ixf"))
            nc.vector.tensor_copy(out=ixf[:], in_=pt[:])
        else:
            ixf = None
        idxTf.append(ixf)
    # transposed segment ids and segment-start mask (fp32)
    jT = chgT = None
    if pairs:
        jT = sp.tile([NF, NP], F32, tag=tg("jT"))
        chgT = sp.tile([NF, NP], F32, tag=tg("chgT"))
        for src, dst in ((jseg, jT), (chg, chgT)):
            pt = pp.tile([NF, NP], F32, space="PSUM", tag="pt_shared")
            nc.tensor.transpose(out=pt[:], in_=src[:], identity=ident[:])
            nc.vector.tensor_copy(out=dst[:], in_=pt[:])
    return idxT, idxTf, jT, chgT, (c0, c1, c2)


def _build_body(nc, sp, op_, pp, ids, emb0, emb1, emb2, pos, fc, rep,
                do_scan=True, gathers=(0, 1, 2), store=True, gather_cols=1,
                pe_pairs=False, bis_ohs=True, bis_ctj=True, bis_ohj=True,
                bis_mm=True, bis_add=True, sep_gathers=False, dma_pairs=False):
    tagp = ""  # shared tags across reps -> reps reuse the same SBUF slots
    tokI = sp.tile([NP, NF], I32, tag=f"{tagp}tokI")
    nc.sync.dma_start(out=tokI[:], in_=ids.ap()[:])

    if do_scan:
        idxT, idxTf, jT, chgT, (c0, c1, c2) = _scan_phase(nc, sp, pp, tokI,
                                                          tagp)
        # final counters: c0/c1/c2 at token 4095 = [31, 127]
        # (DMA has no partition-start limits; cast to int32 on host)
        for j, c in enumerate((c0, c1, c2)):
            nc.sync.dma_start(
                out=fc.ap()[0:1, j : j + 1], in_=c[31:32, NF - 1 : NF]
            )
        if not gathers and not store and not pe_pairs:
            return
    else:
        # ablation: fake indices derived trivially from tokens
        ix = sp.tile([NF, NP], I32, tag="fakeix")
        nc.gpsimd.iota(ix[:], pattern=[[0, NP]], base=0, channel_multiplier=1)
        ixf = sp.tile([NF, NP], F32, tag="fakeixf")
        nc.vector.tensor_copy(out=ixf[:], in_=ix[:])
        idxT = [ix, ix, ix]
        idxTf = [ixf, ixf, ixf]
        jT = chgT = ixf

    if pe_pairs or dma_pairs:
        identF = sp.tile([NF, NF], F32, tag="identF")
        make_identity(nc, identF[:])
        iotaI = sp.tile([NF, 1], I32, tag="iotaI")
        nc.gpsimd.iota(iotaI[:], pattern=[[0, 1]], base=0, channel_multiplier=1)
        iotaF = sp.tile([NF, 1], F32, tag="iotaF")
        nc.vector.tensor_copy(out=iotaF[:], in_=iotaI[:])
        iotaRowI = sp.tile([NF, NF], I32, tag="iotaRowI")
        nc.gpsimd.iota(iotaRowI[:], pattern=[[1, NF]], base=0,
                       channel_multiplier=0)
        iotaRow = sp.tile([NF, NF], F32, tag="iotaRow")
        nc.vector.tensor_copy(out=iotaRow[:], in_=iotaRowI[:])

        # scatter (c1,c2) of each segment's first token into pc[seg] via 32
        # accumulating one-hot matmuls (exact in fp32). The two accumulation
        # groups sit in different PSUM banks (cols 0 and 512); the tile
        # shares the scan-transpose slot (sequential lifetimes).
        pc = pp.tile([NF, D], F32, space="PSUM", tag="pt_shared")
        for b in range(NP):
            ohs = sp.tile([NF, NF], F32, tag=f"ohs{b % 2}")
            if bis_ohs:
                nc.vector.scalar_tensor_tensor(
                    out=ohs[:], in0=iotaRow[:], scalar=jT[:, b : b + 1],
                    in1=chgT[:, b : b + 1].to_broadcast([NF, NF]),
                    op0=ALU.is_equal, op1=ALU.mult,
                )
            else:
                nc.vector.tensor_copy(out=ohs[:], in_=iotaRow[:])
            nc.tensor.matmul(pc[:, 0:1], lhsT=ohs[:],
                             rhs=idxTf[1][:, b : b + 1],
                             start=(b == 0), stop=(b == NP - 1))
            nc.tensor.matmul(pc[:, 512:513], lhsT=ohs[:],
                             rhs=idxTf[2][:, b : b + 1],
                             start=(b == 0), stop=(b == NP - 1))
        pcI = sp.tile([NF, 2], I32, tag="pcI")
        nc.vector.tensor_copy(out=pcI[:, 0:1], in_=pc[:, 0:1])
        nc.vector.tensor_copy(out=pcI[:, 1:2], in_=pc[:, 512:513])
        # pair table: pairsum[j, :] = emb1[pc1_j] + emb2[pc2_j]
        pairsum = sp.tile([NF, D], F32, tag="pairsum")
        nc.gpsimd.indirect_dma_start(
            out=pairsum[:], out_offset=None, in_=emb1.ap()[:],
            in_offset=bass.IndirectOffsetOnAxis(ap=pcI[:, 0:1], axis=0),
        )
        nc.gpsimd.indirect_dma_start(
            out=pairsum[:], out_offset=None, in_=emb2.ap()[:],
            in_offset=bass.IndirectOffsetOnAxis(ap=pcI[:, 1:2], axis=0),
            compute_op=ALU.add,
        )

    if dma_pairs:
        # pair table to DRAM; per-token pair lookup then rides the gather DMA
        pairsum_d = nc.dram_tensor(f"pairsum_d{rep}", [NF, D], F32)
        nc.sync.dma_start(out=pairsum_d.ap()[:], in_=pairsum[:])
        jI = sp.tile([NF, NP], I32, tag="jI")
        nc.vector.tensor_copy(out=jI[:], in_=jT[:])

    if pe_pairs:
        # Phase 1: build all 32 segment one-hot lhsT tiles up front (PE
        # transpose-broadcast -> DVE compare), so the block loop below has no
        # DVE<->PE ping-pong on its critical path.
        ohjS = []
        for b in range(NP):
            ctj = _build_body._ct_pool.tile([NF, NF], F32, space="PSUM",
                                            tag="ctj")
            nc.tensor.transpose(
                out=ctj[:], in_=jT[:, b : b + 1].to_broadcast([NF, NF]),
                identity=identF[:],
            )
            oh = sp.tile([NF, NF], F32, tag=f"ohjS{b}")
            nc.vector.tensor_tensor(
                out=oh[:], in0=ctj[:],
                in1=iotaF[:].to_broadcast([NF, NF]), op=ALU.is_equal,
            )
            ohjS.append(oh)

    tabs = (emb0, emb1, emb2)
    k = gather_cols

    if sep_gathers:
        # each table gathered into its own tile (independent SWDGE streams),
        # summed on DVE, stored
        for b in range(NP):
            ts_ = []
            for j in (0, 1, 2):
                tj = op_.tile([NF, D], F32, tag=f"sg{j}")
                nc.gpsimd.indirect_dma_start(
                    out=tj[:], out_offset=None, in_=tabs[j].ap()[:],
                    in_offset=bass.IndirectOffsetOnAxis(
                        ap=idxT[j][:, b : b + 1], axis=0),
                )
                ts_.append(tj)
            nc.vector.tensor_tensor(out=ts_[0][:], in0=ts_[0][:],
                                    in1=ts_[1][:], op=ALU.add)
            nc.vector.tensor_tensor(out=ts_[0][:], in0=ts_[0][:],
                                    in1=ts_[2][:], op=ALU.add)
            if store:
                nc.sync.dma_start(
                    out=pos.ap()[b * NF : (b + 1) * NF, :], in_=ts_[0][:]
                )
        return

    for b in range(0, NP, k):
        t = op_.tile([NF, k * D], F32, tag="out")
        first = True
        for j in gathers:
            nc.gpsimd.indirect_dma_start(
                out=t[:], out_offset=None, in_=tabs[j].ap()[:],
                in_offset=bass.IndirectOffsetOnAxis(
                    ap=idxT[j][:, b : b + k], axis=0),
                compute_op=ALU.bypass if first else ALU.add,
            )
            first = False
        if dma_pairs:
            nc.gpsimd.indirect_dma_start(
                out=t[:], out_offset=None, in_=pairsum_d.ap()[:],
                in_offset=bass.IndirectOffsetOnAxis(
                    ap=jI[:, b : b + 1], axis=0),
                compute_op=ALU.bypass if first else ALU.add,
            )
            first = False

        if first and not pe_pairs:
            nc.vector.memset(t[:], 0.0)

        if pe_pairs:
            # broadcast segment ids across partitions via PE transpose, build
            # the segment one-hot lhsT on DVE, then po = pairsum[jseg_tok]
            po = _build_body._po_pool.tile([NF, D], F32, space="PSUM", tag="po")
            for h in range(0, D, 512):
                sl = slice(h, h + 512)
                nc.tensor.matmul(po[:, sl], lhsT=ohjS[b][:],
                                 rhs=pairsum[:, sl],
                                 start=True, stop=True)
            if gathers:
                nc.any.tensor_tensor(
                    out=t[:], in0=t[:], in1=po[:], op=ALU.add
                )
            else:
                nc.any.tensor_copy(out=t[:], in_=po[:])

        if store:
            if k == 1:
                nc.sync.dma_start(
                    out=pos.ap()[b * NF : (b + 1) * NF, :], in_=t[:]
                )
            else:
                dst = pos.ap()[b * NF : (b + k) * NF, :].rearrange(
                    "(k p) d -> p k d", p=NF
                )
                nc.sync.dma_start(
                    out=dst, in_=t[:].rearrange("p (k d) -> p k d", k=k)
                )


def build(out_bufs=10, reps=1, **variant):
    nc = bacc.Bacc("TRN2", target_bir_lowering=False, debug=False,
                   num_devices=NCORES)
    ids = nc.dram_tensor("ids", [NP, NF], I32, kind="ExternalInput")
    emb0 = nc.dram_tensor("emb0", [V0, D], F32, kind="ExternalInput")
    emb1 = nc.dram_tensor("emb1", [V1, D], F32, kind="ExternalInput")
    emb2 = nc.dram_tensor("emb2", [V2, D], F32, kind="ExternalInput")
    pos = nc.dram_tensor("pos", [S, D], F32, kind="ExternalOutput")
    fc = nc.dram_tensor("fc", [1, 3], F32, kind="ExternalOutput")

    with TileContext(nc) as tc:
        with (
            tc.tile_pool(name="scan", bufs=1) as sp,
            tc.tile_pool(name="outp", bufs=out_bufs) as op_,
            tc.tile_pool(name="psum", bufs=1, space="PSUM") as pp,
            tc.tile_pool(name="psum_po", bufs=2, space="PSUM") as pp2,
            tc.tile_pool(name="psum_ct", bufs=2, space="PSUM") as pp3,
        ):
            _build_body._po_pool = pp2
            _build_body._ct_pool = pp3
            for rep in range(reps):
                _build_body(nc, sp, op_, pp, ids, emb0, emb1, emb2, pos, fc,
                            rep, **variant)

    nc.compile()
    return nc


_NC_CACHE = {}

BEST_VARIANT = dict(gathers=(0,), pe_pairs=True)
FALLBACK_VARIANT = dict()  # all-DMA gather path, no segment cap


def _get_nc(fallback=False):
    key = "fb" if fallback else "best"
    if key not in _NC_CACHE:
        _NC_CACHE[key] = build(**(FALLBACK_VARIANT if fallback
                                  else BEST_VARIANT))
    return _NC_CACHE[key]


def _max_segments(ids):
    """Host replica of the segment count: the pe_pairs path supports at most
    128 (c1,c2)-segments per sequence."""
    mx = 0
    for tok in ids:
        e = (tok == 1)
        dn = np.maximum.accumulate(e)
        m = ~dn
        k1 = (tok == 10) & m
        k2 = (tok == 11) & m
        i0 = (~(tok == 10)) & (~(tok == 11)) & m
        cum0p = 2 + np.cumsum(i0)
        evA = k1 | k2
        Ap = np.maximum.accumulate(cum0p * evA)
        c0 = (cum0p - Ap) % 514
        carry0 = (c0 == 0) & i0
        nseg = int(k1.sum() + k2.sum() + carry0.sum() + e.any())
        mx = max(mx, nseg)
    return mx


def kernel(input_ids, emb0, emb1, emb2, _nc=None):
    ids = np.ascontiguousarray(np.asarray(input_ids).astype(np.int32))
    emb0 = np.ascontiguousarray(np.asarray(emb0, dtype=np.float32))
    emb1 = np.ascontiguousarray(np.asarray(emb1, dtype=np.float32))
    emb2 = np.ascontiguousarray(np.asarray(emb2, dtype=np.float32))
    assert ids.shape == (NCORES, S)

    if _nc is not None:
        nc = _nc
    else:
        nc = _get_nc(fallback=_max_segments(ids) > NF - 1)
    in_maps = [
        {
            "ids": ids[i].reshape(NP, NF),
            "emb0": emb0,
            "emb1": emb1,
            "emb2": emb2,
        }
        for i in range(NCORES)
    ]
    res = run_bass_kernel_spmd(nc, in_maps, core_ids=list(range(NCORES)))
    pos = np.stack([res.results[i]["pos"] for i in range(NCORES)])
    fc = np.stack([res.results[i]["fc"].reshape(3) for i in range(NCORES)])
    fc = np.rint(fc).astype(np.int32)
    return pos, fc
